# revision 19
# baseline (speedup 1.0000x reference)
"""DarkChannel Trainium2 kernel.

Computes, per image: channel-min over C=3, then 15x15 sliding-window min
with reflect padding (== clamped-window min, since reflected indices always
fall inside the clamped window), over [B,3,512,512] f32 -> [B,1,512,512].

Sharding: pure data parallel, batch 16 -> 2 images on each of 8 cores.

Numerics: all min-chains run in bf16. Rounding to bf16 is monotone, and min
commutes with monotone rounding, so the result equals bf16_round(true dark
channel): max rel err ~2^-9 (inputs are in [0,1)), far inside the 2e-2 gate.
bf16 doubles DVE TensorTensor throughput (2x_1p mode: the only DVE fast mode
InstTensorTensor supports) and makes PE transposes 2x faster per the cost
model (1 cycle/row vs 2).

Per image (per core):
  1. DMA 3 channel planes f32 into SBUF (rows on partitions).
  2. f32->bf16 casts split across the otherwise-idle Act + Pool engines
     (DVE is the bottleneck; TT min is DVE-only in this walrus build --
     Pool TensorTensor and CCE-min DMAs are rejected).
  3. DVE: chan-min (2 bf16 TT) into a 526-wide padded buffer (borders
     preset to BIG; clamped-window erosion == erosion with +inf pad).
  4. Horizontal 15-tap min via log-doubling shifts (1,2,4,7): 4 bf16 TT.
  5. Transpose 512x512 via TensorE (16 bf16 128x128 blocks) into per-col-tile
     PSUM tiles that are pre-padded with BIG borders; the first vertical
     pass TT-reads PSUM directly (saves an Act PSUM->SBUF copy).
  6. Vertical passes 2-4 on SBUF, transpose back, Act copies PSUM->SBUF
     with bf16->f32 cast, DMA out.
"""

import numpy as np

import concourse.bacc as bacc
import concourse.mybir as mybir
from concourse.tile import TileContext
from concourse.masks import make_identity
from concourse.bass_utils import run_bass_kernel_spmd

F32 = mybir.dt.float32
BF16 = mybir.dt.bfloat16
MIN = mybir.AluOpType.min

P = 128          # SBUF partitions
H = W = 512
NT = 4           # row-tiles (128 rows each) per image
PAD = 7
PW = W + 2 * PAD  # 526
BIG = 1.0e30
B_PER_CORE = 2
N_CORES = 8


def _build(repeat=1, n_images=B_PER_CORE, ngrp=1, vgrp=2, conv_split=2,
           pool_chans=(2,), split_load=True, split_store=True,
           xin_bufs=2, xbf_bufs=2, work_bufs=4, hmin_bufs=2, vwork_bufs=4,
           out_bufs=2, vps_bufs=2, tout_bufs=2, pool_psum_memset=True,
           pool_h_memset=True, group_vpsum=True):
    """Build + compile the Bacc program. Returns nc.

    ngrp: row-groups for chan-min + horizontal chain. vgrp: col-groups for
    the vertical chain (passes 2-4). conv_split: row-halves per cast op.
    pool_chans: channel indices cast on the Pool engine (rest on Act).
    group_vpsum: one PSUM tile per col-group with 1024-elem (1-bank) stride
    per col-tile, so V pass 1 is a single grouped TT from PSUM."""
    tpg = NT // ngrp
    tpv = NT // vgrp
    nc = bacc.Bacc("TRN2", target_bir_lowering=False, debug=False)
    x = nc.declare_dram_parameter("x", [n_images, 3, H, W], F32, isOutput=False)
    y = nc.declare_dram_parameter("y", [n_images, 1, H, W], F32, isOutput=True)

    with TileContext(nc) as tc:
        with (
            tc.tile_pool(name="consts", bufs=1) as consts,
            tc.tile_pool(name="xin", bufs=xin_bufs) as xin_pool,
            tc.tile_pool(name="xbf", bufs=xbf_bufs) as xbf_pool,
            tc.tile_pool(name="tmp", bufs=2) as tmp_pool,
            tc.tile_pool(name="work", bufs=work_bufs) as work_pool,
            tc.tile_pool(name="hmin", bufs=hmin_bufs) as hmin_pool,
            tc.tile_pool(name="vwork", bufs=vwork_bufs) as vwork_pool,
            tc.tile_pool(name="outp", bufs=out_bufs) as out_pool,
            tc.tile_pool(name="vps", bufs=vps_bufs, space="PSUM") as vps_pool,
            tc.tile_pool(name="tout", bufs=tout_bufs, space="PSUM") as tout_pool,
        ):
            ident = consts.tile([P, P], BF16)
            make_identity(nc, ident)

            for _rep in range(repeat):
                for b in range(n_images):
                    # ---- load: 3 channel planes f32, rows->partitions ----
                    X = xin_pool.tile([P, 3, NT, W], F32, tag="xin")
                    xr = x[b].rearrange("c (i p) w -> p c i w", p=P)
                    if split_load:
                        # one DMA per (half, channel): first half's channels
                        # arrive first -> casts/chan-min start early
                        for hlf in range(2):
                            i0, i1 = hlf * 2, hlf * 2 + 2
                            for c in range(3):
                                nc.sync.dma_start(
                                    out=X[:, c, i0:i1], in_=xr[:, c, i0:i1]
                                )
                    else:
                        for c in range(3):
                            nc.sync.dma_start(out=X[:, c], in_=xr[:, c])

                    # ---- f32 -> bf16 casts on Act (+ Pool) ----
                    Xb = xbf_pool.tile([P, 3, NT, W], BF16, tag="xbf")
                    tph = NT // conv_split
                    for hlf in range(conv_split):
                        i0, i1 = hlf * tph, (hlf + 1) * tph
                        for c in range(3):
                            eng = nc.gpsimd if c in pool_chans else nc.scalar
                            if eng is nc.gpsimd:
                                eng.tensor_copy(out=Xb[:, c, i0:i1],
                                                in_=X[:, c, i0:i1])
                            else:
                                eng.copy(out=Xb[:, c, i0:i1], in_=X[:, c, i0:i1])

                    # ---- chan-min (bf16) into padded buffer + H chain ----
                    hmins = []
                    for g in range(ngrp):
                        t0, t1 = g * tpg, (g + 1) * tpg
                        T = tmp_pool.tile([P, tpg, W], BF16, tag="tmp")
                        nc.vector.tensor_tensor(
                            out=T[:], in0=Xb[:, 0, t0:t1],
                            in1=Xb[:, 1, t0:t1], op=MIN,
                        )
                        Pb = work_pool.tile([P, tpg, PW], BF16, tag="work")
                        hms = nc.gpsimd if pool_h_memset else nc.vector
                        hms.memset(Pb[:, :, 0:PAD], BIG)
                        hms.memset(Pb[:, :, PAD + W:PW], BIG)
                        nc.vector.tensor_tensor(
                            out=Pb[:, :, PAD:PAD + W], in0=T[:],
                            in1=Xb[:, 2, t0:t1], op=MIN,
                        )
                        # H chain: shifts 1,2,4,7 (windows 2,4,8,15)
                        cur, wid = Pb, PW
                        for s in (1, 2, 4):
                            nw = wid - s
                            nxt = work_pool.tile([P, tpg, PW], BF16, tag="work")
                            nc.vector.tensor_tensor(
                                out=nxt[:, :, 0:nw], in0=cur[:, :, 0:nw],
                                in1=cur[:, :, s:s + nw], op=MIN,
                            )
                            cur, wid = nxt, nw
                        hm = hmin_pool.tile([P, tpg, W], BF16, tag="hmin")
                        nc.vector.tensor_tensor(
                            out=hm[:], in0=cur[:, :, 0:W],
                            in1=cur[:, :, 7:7 + W], op=MIN,
                        )
                        hmins.append(hm)

                    # ---- transpose -> PSUM (two 1024B col-tile slots per
                    # 2KB bank, 4B-aligned writes); Act copies each group
                    # into a BIG-padded SBUF buffer (a TT cannot read both
                    # inputs from PSUM, so the V chain runs from SBUF) ----
                    vmins = []
                    for g in range(vgrp):
                        VP = vps_pool.tile([P, tpv, W], BF16, tag="vp")
                        for jj in range(tpv):
                            j = g * tpv + jj  # absolute col-tile
                            for i in range(NT):  # row-tile
                                hg = hmins[i // tpg]
                                nc.tensor.transpose(
                                    VP[:, jj, i * P:(i + 1) * P],
                                    hg[:, i % tpg, j * P:(j + 1) * P],
                                    ident,
                                )
                        Vb = vwork_pool.tile([P, tpv, PW], BF16, tag="vwork")
                        vms = nc.gpsimd if pool_h_memset else nc.vector
                        vms.memset(Vb[:, :, 0:PAD], BIG)
                        vms.memset(Vb[:, :, PAD + W:PW], BIG)
                        nc.scalar.copy(out=Vb[:, :, PAD:PAD + W], in_=VP[:])
                        # V chain on SBUF: shifts 1,2,4,7
                        cur, wid = Vb, PW
                        for s in (1, 2, 4):
                            nw = wid - s
                            nxt = vwork_pool.tile([P, tpv, PW], BF16,
                                                  tag="vwork")
                            nc.vector.tensor_tensor(
                                out=nxt[:, :, 0:nw], in0=cur[:, :, 0:nw],
                                in1=cur[:, :, s:s + nw], op=MIN,
                            )
                            cur, wid = nxt, nw
                        vm = vwork_pool.tile([P, tpv, W], BF16, tag="vmin")
                        nc.vector.tensor_tensor(
                            out=vm[:], in0=cur[:, :, 0:W],
                            in1=cur[:, :, 7:7 + W], op=MIN,
                        )
                        vmins.append(vm)

                    # ---- transpose back + cast to f32 + store ----
                    yr = y[b, 0].rearrange("(i p) w -> p i w", p=P)
                    OUT = out_pool.tile([P, NT, W], F32, tag="outp")
                    for i in range(NT):
                        TO = tout_pool.tile([P, W], BF16, tag="to")
                        for j in range(NT):
                            vg = vmins[j // tpv]
                            nc.tensor.transpose(
                                TO[:, j * P:(j + 1) * P],
                                vg[:, j % tpv, i * P:(i + 1) * P],
                                ident,
                            )
                        nc.scalar.copy(out=OUT[:, i], in_=TO[:])
                        if split_store:
                            nc.sync.dma_start(out=yr[:, i], in_=OUT[:, i])
                    if not split_store:
                        nc.sync.dma_start(out=yr, in_=OUT[:])
    nc.compile()
    return nc


_CACHE = {}


def _get_nc(**kw):
    key = tuple(sorted(kw.items()))
    if key not in _CACHE:
        _CACHE[key] = _build(**kw)
    return _CACHE[key]


def kernel(x: np.ndarray) -> np.ndarray:
    """Full-input entry point: x [16,3,512,512] f32 -> [16,1,512,512] f32."""
    x = np.ascontiguousarray(x, dtype=np.float32)
    B = x.shape[0]
    assert B == N_CORES * B_PER_CORE, x.shape
    nc = _get_nc()
    in_maps = [
        {"x": x[c * B_PER_CORE:(c + 1) * B_PER_CORE]} for c in range(N_CORES)
    ]
    res = run_bass_kernel_spmd(nc, in_maps, core_ids=list(range(N_CORES)))
    out = np.concatenate([res.results[c]["y"] for c in range(N_CORES)], axis=0)
    return out.astype(np.float32, copy=False)


# revision 31
# speedup vs baseline: 1.4165x; 1.4165x over previous
"""DarkChannel Trainium2 kernel.

Computes, per image: channel-min over C=3, then 15x15 sliding-window min
with reflect padding (== clamped-window min, since reflected indices always
fall inside the clamped window), over [B,3,512,512] f32 -> [B,1,512,512].

Sharding: pure data parallel, batch 16 -> 2 images on each of 8 cores.

Numerics: all min-chains run in bf16. Rounding to bf16 is monotone, and min
commutes with monotone rounding, so the result equals bf16_round(true dark
channel): max rel err ~2^-9 (inputs are in [0,1)), far inside the 2e-2 gate.
bf16 doubles DVE TensorTensor throughput (2x_1p mode: the only DVE fast mode
InstTensorTensor supports) and makes PE transposes 2x faster (1 cycle/row).

Per image (per core):
  1. DMA 3 channel planes f32 into SBUF (rows on partitions), split per
     (row-half, channel) so early halves overlap compute.
  2. f32->bf16 casts on the Act engine (DVE is the bottleneck; TT min is
     DVE-only in this walrus build -- Pool TensorTensor and CCE-min DMAs
     are rejected; Pool casts are slower and sit on the critical path).
  3. DVE: chan-min (2 bf16 TT) into a 526-wide padded buffer (borders BIG
     via Pool memset; clamped-window erosion == erosion with +inf pad).
  4. Horizontal 15-tap min via log-doubling shifts (1,2,4,7): 4 bf16 TT.
  5. Transpose 512x512 via TensorE (16 bf16 128x128 blocks per direction)
     into PSUM (bf16 PSUM writes are legal only from Matmult/Memset and
     must stay 4B-aligned inside one 2KB bank -> two 512-elem col-tile
     slots per bank tile). Act copies PSUM->SBUF into a BIG-padded buffer
     (a TT cannot read both inputs from PSUM).
  6. Vertical chain (4 bf16 TT) per col-group, transpose back, Act copies
     PSUM->SBUF with bf16->f32 cast, DMA out per row-tile.

Steady-state model (TimelineSim): DVE 23.3 us/body, DMA 23.3 us/body
(8.39 MB @ ~360 GB/s) -- both at roofline; body ~24.6 us vs 52.2 us for
the f32 baseline. The program's first image uses finer row groups
(first_ngrp) to shorten the single-exec leading edge.
"""

import numpy as np

import concourse.bacc as bacc
import concourse.mybir as mybir
from concourse.tile import TileContext
from concourse.masks import make_identity
from concourse.bass_utils import run_bass_kernel_spmd

F32 = mybir.dt.float32
BF16 = mybir.dt.bfloat16
MIN = mybir.AluOpType.min

P = 128          # SBUF partitions
H = W = 512
NT = 4           # row-tiles (128 rows each) per image
PAD = 7
PW = W + 2 * PAD  # 526
BIG = 1.0e30
B_PER_CORE = 2
N_CORES = 8


def _build(repeat=1, n_images=B_PER_CORE, ngrp=1, vgrp=2, conv_split=1,
           pool_chans=(), split_load=True, split_store=True,
           xin_bufs=2, xbf_bufs=2, work_bufs=4, hmin_bufs=2, vwork_bufs=8,
           out_bufs=2, vps_bufs=3, tout_bufs=3,
           pool_h_memset=True, first_ngrp=2, store_eng="sync"):
    """Build + compile the Bacc program. Returns nc.

    ngrp: row-groups for chan-min + horizontal chain (first_ngrp overrides
    for the program's first image to shorten the leading edge). vgrp:
    col-groups for the vertical chain. conv_split: row-splits per cast op.
    pool_chans: channel indices cast on the Pool engine (rest on Act)."""
    tpv = NT // vgrp
    nc = bacc.Bacc("TRN2", target_bir_lowering=False, debug=False)
    x = nc.declare_dram_parameter("x", [n_images, 3, H, W], F32, isOutput=False)
    y = nc.declare_dram_parameter("y", [n_images, 1, H, W], F32, isOutput=True)

    with TileContext(nc) as tc:
        with (
            tc.tile_pool(name="consts", bufs=1) as consts,
            tc.tile_pool(name="xin", bufs=xin_bufs) as xin_pool,
            tc.tile_pool(name="xbf", bufs=xbf_bufs) as xbf_pool,
            tc.tile_pool(name="tmp", bufs=2) as tmp_pool,
            tc.tile_pool(name="work", bufs=work_bufs) as work_pool,
            tc.tile_pool(name="hmin", bufs=hmin_bufs) as hmin_pool,
            tc.tile_pool(name="vwork", bufs=vwork_bufs) as vwork_pool,
            tc.tile_pool(name="outp", bufs=out_bufs) as out_pool,
            tc.tile_pool(name="vps", bufs=vps_bufs, space="PSUM") as vps_pool,
            tc.tile_pool(name="tout", bufs=tout_bufs, space="PSUM") as tout_pool,
        ):
            ident = consts.tile([P, P], BF16)
            make_identity(nc, ident)

            for _rep in range(repeat):
                for b in range(n_images):
                    # finer row-groups for the program's first image: the
                    # chan-min + H chain start after half the plane loads,
                    # shortening the single-exec leading edge; steady-state
                    # bodies use the coarser (lower-overhead) grouping.
                    grp = first_ngrp if (_rep == 0 and b == 0 and first_ngrp) \
                        else ngrp
                    tpg = NT // grp
                    # ---- load: 3 channel planes f32, rows->partitions ----
                    X = xin_pool.tile([P, 3, NT, W], F32, tag="xin")
                    xr = x[b].rearrange("c (i p) w -> p c i w", p=P)
                    if split_load == "chan":
                        # channel-major: plane c completes after (c+1)/3 of
                        # the load -> whole-plane casts pipeline with the DMA
                        for c in range(3):
                            for hlf in range(2):
                                i0, i1 = hlf * 2, hlf * 2 + 2
                                nc.sync.dma_start(
                                    out=X[:, c, i0:i1], in_=xr[:, c, i0:i1]
                                )
                    elif split_load:
                        # one DMA per (half, channel): first half's channels
                        # arrive first -> half-plane casts/chan-min start early
                        for hlf in range(2):
                            i0, i1 = hlf * 2, hlf * 2 + 2
                            for c in range(3):
                                nc.sync.dma_start(
                                    out=X[:, c, i0:i1], in_=xr[:, c, i0:i1]
                                )
                    else:
                        for c in range(3):
                            nc.sync.dma_start(out=X[:, c], in_=xr[:, c])

                    # ---- f32 -> bf16 casts on Act (+ Pool) ----
                    Xb = xbf_pool.tile([P, 3, NT, W], BF16, tag="xbf")
                    csp = max(conv_split, grp)  # first image: match groups
                    tph = NT // csp
                    for hlf in range(csp):
                        i0, i1 = hlf * tph, (hlf + 1) * tph
                        for c in range(3):
                            eng = nc.gpsimd if c in pool_chans else nc.scalar
                            if eng is nc.gpsimd:
                                eng.tensor_copy(out=Xb[:, c, i0:i1],
                                                in_=X[:, c, i0:i1])
                            else:
                                eng.copy(out=Xb[:, c, i0:i1], in_=X[:, c, i0:i1])

                    # ---- chan-min (bf16) into padded buffer + H chain ----
                    hmins = []
                    for g in range(grp):
                        t0, t1 = g * tpg, (g + 1) * tpg
                        T = tmp_pool.tile([P, tpg, W], BF16, tag=f"tmp{grp}")
                        nc.vector.tensor_tensor(
                            out=T[:], in0=Xb[:, 0, t0:t1],
                            in1=Xb[:, 1, t0:t1], op=MIN,
                        )
                        Pb = work_pool.tile([P, tpg, PW], BF16, tag=f"work{grp}")
                        hms = nc.gpsimd if pool_h_memset else nc.vector
                        hms.memset(Pb[:, :, 0:PAD], BIG)
                        hms.memset(Pb[:, :, PAD + W:PW], BIG)
                        nc.vector.tensor_tensor(
                            out=Pb[:, :, PAD:PAD + W], in0=T[:],
                            in1=Xb[:, 2, t0:t1], op=MIN,
                        )
                        # H chain: shifts 1,2,4,7 (windows 2,4,8,15)
                        cur, wid = Pb, PW
                        for s in (1, 2, 4):
                            nw = wid - s
                            nxt = work_pool.tile([P, tpg, PW], BF16,
                                                 tag=f"work{grp}")
                            nc.vector.tensor_tensor(
                                out=nxt[:, :, 0:nw], in0=cur[:, :, 0:nw],
                                in1=cur[:, :, s:s + nw], op=MIN,
                            )
                            cur, wid = nxt, nw
                        hm = hmin_pool.tile([P, tpg, W], BF16, tag=f"hmin{grp}")
                        nc.vector.tensor_tensor(
                            out=hm[:], in0=cur[:, :, 0:W],
                            in1=cur[:, :, 7:7 + W], op=MIN,
                        )
                        hmins.append(hm)

                    # ---- transpose -> PSUM (two 1024B col-tile slots per
                    # 2KB bank, 4B-aligned writes); Act copies each group
                    # into a BIG-padded SBUF buffer (a TT cannot read both
                    # inputs from PSUM, so the V chain runs from SBUF) ----
                    vmins = []
                    for g in range(vgrp):
                        VP = vps_pool.tile([P, tpv, W], BF16, tag="vp")
                        for jj in range(tpv):
                            j = g * tpv + jj  # absolute col-tile
                            for i in range(NT):  # row-tile
                                hg = hmins[i // tpg]
                                nc.tensor.transpose(
                                    VP[:, jj, i * P:(i + 1) * P],
                                    hg[:, i % tpg, j * P:(j + 1) * P],
                                    ident,
                                )
                        Vb = vwork_pool.tile([P, tpv, PW], BF16, tag="vwork")
                        vms = nc.gpsimd if pool_h_memset else nc.vector
                        vms.memset(Vb[:, :, 0:PAD], BIG)
                        vms.memset(Vb[:, :, PAD + W:PW], BIG)
                        nc.scalar.copy(out=Vb[:, :, PAD:PAD + W], in_=VP[:])
                        # V chain on SBUF: shifts 1,2,4,7
                        cur, wid = Vb, PW
                        for s in (1, 2, 4):
                            nw = wid - s
                            nxt = vwork_pool.tile([P, tpv, PW], BF16,
                                                  tag="vwork")
                            nc.vector.tensor_tensor(
                                out=nxt[:, :, 0:nw], in0=cur[:, :, 0:nw],
                                in1=cur[:, :, s:s + nw], op=MIN,
                            )
                            cur, wid = nxt, nw
                        vm = vwork_pool.tile([P, tpv, W], BF16, tag="vmin")
                        nc.vector.tensor_tensor(
                            out=vm[:], in0=cur[:, :, 0:W],
                            in1=cur[:, :, 7:7 + W], op=MIN,
                        )
                        vmins.append(vm)

                    # ---- transpose back + cast to f32 + store ----
                    # stores issue from the Act engine queue (TRN2 HWDGE
                    # engines are SP + Act) so a blocked input load on SP
                    # can't head-of-line-block the stores.
                    st = nc.scalar if store_eng == "scalar" else nc.sync
                    yr = y[b, 0].rearrange("(i p) w -> p i w", p=P)
                    OUT = out_pool.tile([P, NT, W], F32, tag="outp")
                    for i in range(NT):
                        TO = tout_pool.tile([P, W], BF16, tag="to")
                        for j in range(NT):
                            vg = vmins[j // tpv]
                            nc.tensor.transpose(
                                TO[:, j * P:(j + 1) * P],
                                vg[:, j % tpv, i * P:(i + 1) * P],
                                ident,
                            )
                        nc.scalar.copy(out=OUT[:, i], in_=TO[:])
                        if split_store:
                            st.dma_start(out=yr[:, i], in_=OUT[:, i])
                    if not split_store:
                        st.dma_start(out=yr, in_=OUT[:])
    nc.compile()
    return nc


_CACHE = {}


def _get_nc(**kw):
    key = tuple(sorted(kw.items()))
    if key not in _CACHE:
        _CACHE[key] = _build(**kw)
    return _CACHE[key]


def kernel(x: np.ndarray) -> np.ndarray:
    """Full-input entry point: x [16,3,512,512] f32 -> [16,1,512,512] f32."""
    x = np.ascontiguousarray(x, dtype=np.float32)
    B = x.shape[0]
    assert B == N_CORES * B_PER_CORE, x.shape
    nc = _get_nc()
    in_maps = [
        {"x": x[c * B_PER_CORE:(c + 1) * B_PER_CORE]} for c in range(N_CORES)
    ]
    res = run_bass_kernel_spmd(nc, in_maps, core_ids=list(range(N_CORES)))
    out = np.concatenate([res.results[c]["y"] for c in range(N_CORES)], axis=0)
    return out.astype(np.float32, copy=False)


# revision 33
# speedup vs baseline: 1.6464x; 1.1622x over previous
"""DarkChannel Trainium2 kernel.

Computes, per image: channel-min over C=3, then 15x15 sliding-window min
with reflect padding (== clamped-window min, since reflected indices always
fall inside the clamped window), over [B,3,512,512] f32 -> [B,1,512,512].

Sharding: pure data parallel, batch 16 -> 2 images on each of 8 cores.

Numerics: all min-chains run in bf16. Rounding to bf16 is monotone, and min
commutes with monotone rounding, so the result equals bf16_round(true dark
channel): max rel err ~2^-9 (inputs are in [0,1)), far inside the 2e-2 gate.
bf16 doubles DVE TensorTensor throughput (2x_1p mode: the only DVE fast mode
InstTensorTensor supports) and makes PE transposes 2x faster (1 cycle/row).

Per image (per core):
  1. DMA 3 channel planes f32 into SBUF (rows on partitions), split per
     (row-half, channel) so early halves overlap compute.
  2. f32->bf16 casts on the Act engine (DVE is the bottleneck; TT min is
     DVE-only in this walrus build -- Pool TensorTensor and CCE-min DMAs
     are rejected; Pool casts are slower and sit on the critical path).
  3. DVE: chan-min (2 bf16 TT) into a 526-wide padded buffer (borders BIG
     via Pool memset; clamped-window erosion == erosion with +inf pad).
  4. Horizontal 15-tap min via log-doubling shifts (1,2,4,7): 4 bf16 TT.
  5. Transpose 512x512 via TensorE (16 bf16 128x128 blocks per direction)
     into PSUM (bf16 PSUM writes are legal only from Matmult/Memset and
     must stay 4B-aligned inside one 2KB bank -> two 512-elem col-tile
     slots per bank tile). Act copies PSUM->SBUF into a BIG-padded buffer
     (a TT cannot read both inputs from PSUM).
  6. Vertical chain (4 bf16 TT) per col-group, transpose back, Act copies
     PSUM->SBUF with bf16->f32 cast, DMA out per row-tile.

Steady-state model (TimelineSim): DVE 23.3 us/body, DMA 23.3 us/body
(8.39 MB @ ~360 GB/s) -- both at roofline; body ~24.6 us vs 52.2 us for
the f32 baseline. The program's first image uses finer row groups
(first_ngrp) to shorten the single-exec leading edge.
"""

import numpy as np

import concourse.bacc as bacc
import concourse.mybir as mybir
from concourse.tile import TileContext
from concourse.masks import make_identity
from concourse.bass_utils import run_bass_kernel_spmd

F32 = mybir.dt.float32
BF16 = mybir.dt.bfloat16
MIN = mybir.AluOpType.min

P = 128          # SBUF partitions
H = W = 512
NT = 4           # row-tiles (128 rows each) per image
PAD = 7
PW = W + 2 * PAD  # 526
BIG = 1.0e30
B_PER_CORE = 2
N_CORES = 8


def _build(repeat=1, n_images=B_PER_CORE, ngrp=1, vgrp=2, conv_split=1,
           pool_chans=(), split_load=True, split_store=True,
           xin_bufs=2, xbf_bufs=2, work_bufs=4, hmin_bufs=2, vwork_bufs=8,
           out_bufs=2, vps_bufs=3, tout_bufs=3,
           pool_h_memset=True, first_ngrp=2, store_eng="scalar",
           act_load_chans=()):
    """Build + compile the Bacc program. Returns nc.

    ngrp: row-groups for chan-min + horizontal chain (first_ngrp overrides
    for the program's first image to shorten the leading edge). vgrp:
    col-groups for the vertical chain. conv_split: row-splits per cast op.
    pool_chans: channel indices cast on the Pool engine (rest on Act)."""
    tpv = NT // vgrp
    nc = bacc.Bacc("TRN2", target_bir_lowering=False, debug=False)
    x = nc.declare_dram_parameter("x", [n_images, 3, H, W], F32, isOutput=False)
    y = nc.declare_dram_parameter("y", [n_images, 1, H, W], F32, isOutput=True)

    with TileContext(nc) as tc:
        with (
            tc.tile_pool(name="consts", bufs=1) as consts,
            tc.tile_pool(name="xin", bufs=xin_bufs) as xin_pool,
            tc.tile_pool(name="xbf", bufs=xbf_bufs) as xbf_pool,
            tc.tile_pool(name="tmp", bufs=2) as tmp_pool,
            tc.tile_pool(name="work", bufs=work_bufs) as work_pool,
            tc.tile_pool(name="hmin", bufs=hmin_bufs) as hmin_pool,
            tc.tile_pool(name="vwork", bufs=vwork_bufs) as vwork_pool,
            tc.tile_pool(name="outp", bufs=out_bufs) as out_pool,
            tc.tile_pool(name="vps", bufs=vps_bufs, space="PSUM") as vps_pool,
            tc.tile_pool(name="tout", bufs=tout_bufs, space="PSUM") as tout_pool,
        ):
            ident = consts.tile([P, P], BF16)
            make_identity(nc, ident)

            for _rep in range(repeat):
                for b in range(n_images):
                    # finer row-groups for the program's first image: the
                    # chan-min + H chain start after half the plane loads,
                    # shortening the single-exec leading edge; steady-state
                    # bodies use the coarser (lower-overhead) grouping.
                    grp = first_ngrp if (_rep == 0 and b == 0 and first_ngrp) \
                        else ngrp
                    tpg = NT // grp
                    # ---- load: 3 channel planes f32, rows->partitions ----
                    X = xin_pool.tile([P, 3, NT, W], F32, tag="xin")
                    xr = x[b].rearrange("c (i p) w -> p c i w", p=P)
                    if split_load == "chan":
                        # channel-major: plane c completes after (c+1)/3 of
                        # the load -> whole-plane casts pipeline with the DMA
                        for c in range(3):
                            for hlf in range(2):
                                i0, i1 = hlf * 2, hlf * 2 + 2
                                nc.sync.dma_start(
                                    out=X[:, c, i0:i1], in_=xr[:, c, i0:i1]
                                )
                    elif split_load:
                        # one DMA per (half, channel): first half's channels
                        # arrive first -> half-plane casts/chan-min start early.
                        # act_load_chans ride the Act HWDGE queue: two queues
                        # measure ~335 GB/s vs ~307 GB/s on one (dmabench).
                        for hlf in range(2):
                            i0, i1 = hlf * 2, hlf * 2 + 2
                            for c in range(3):
                                le = nc.scalar if c in act_load_chans \
                                    else nc.sync
                                le.dma_start(
                                    out=X[:, c, i0:i1], in_=xr[:, c, i0:i1]
                                )
                    else:
                        for c in range(3):
                            nc.sync.dma_start(out=X[:, c], in_=xr[:, c])

                    # ---- f32 -> bf16 casts on Act (+ Pool) ----
                    Xb = xbf_pool.tile([P, 3, NT, W], BF16, tag="xbf")
                    csp = max(conv_split, grp)  # first image: match groups
                    tph = NT // csp
                    for hlf in range(csp):
                        i0, i1 = hlf * tph, (hlf + 1) * tph
                        for c in range(3):
                            eng = nc.gpsimd if c in pool_chans else nc.scalar
                            if eng is nc.gpsimd:
                                eng.tensor_copy(out=Xb[:, c, i0:i1],
                                                in_=X[:, c, i0:i1])
                            else:
                                eng.copy(out=Xb[:, c, i0:i1], in_=X[:, c, i0:i1])

                    # ---- chan-min (bf16) into padded buffer + H chain ----
                    hmins = []
                    for g in range(grp):
                        t0, t1 = g * tpg, (g + 1) * tpg
                        T = tmp_pool.tile([P, tpg, W], BF16, tag=f"tmp{grp}")
                        nc.vector.tensor_tensor(
                            out=T[:], in0=Xb[:, 0, t0:t1],
                            in1=Xb[:, 1, t0:t1], op=MIN,
                        )
                        Pb = work_pool.tile([P, tpg, PW], BF16, tag=f"work{grp}")
                        hms = nc.gpsimd if pool_h_memset else nc.vector
                        hms.memset(Pb[:, :, 0:PAD], BIG)
                        hms.memset(Pb[:, :, PAD + W:PW], BIG)
                        nc.vector.tensor_tensor(
                            out=Pb[:, :, PAD:PAD + W], in0=T[:],
                            in1=Xb[:, 2, t0:t1], op=MIN,
                        )
                        # H chain: shifts 1,2,4,7 (windows 2,4,8,15)
                        cur, wid = Pb, PW
                        for s in (1, 2, 4):
                            nw = wid - s
                            nxt = work_pool.tile([P, tpg, PW], BF16,
                                                 tag=f"work{grp}")
                            nc.vector.tensor_tensor(
                                out=nxt[:, :, 0:nw], in0=cur[:, :, 0:nw],
                                in1=cur[:, :, s:s + nw], op=MIN,
                            )
                            cur, wid = nxt, nw
                        hm = hmin_pool.tile([P, tpg, W], BF16, tag=f"hmin{grp}")
                        nc.vector.tensor_tensor(
                            out=hm[:], in0=cur[:, :, 0:W],
                            in1=cur[:, :, 7:7 + W], op=MIN,
                        )
                        hmins.append(hm)

                    # ---- transpose -> PSUM (two 1024B col-tile slots per
                    # 2KB bank, 4B-aligned writes); Act copies each group
                    # into a BIG-padded SBUF buffer (a TT cannot read both
                    # inputs from PSUM, so the V chain runs from SBUF) ----
                    vmins = []
                    for g in range(vgrp):
                        VP = vps_pool.tile([P, tpv, W], BF16, tag="vp")
                        for jj in range(tpv):
                            j = g * tpv + jj  # absolute col-tile
                            for i in range(NT):  # row-tile
                                hg = hmins[i // tpg]
                                nc.tensor.transpose(
                                    VP[:, jj, i * P:(i + 1) * P],
                                    hg[:, i % tpg, j * P:(j + 1) * P],
                                    ident,
                                )
                        Vb = vwork_pool.tile([P, tpv, PW], BF16, tag="vwork")
                        vms = nc.gpsimd if pool_h_memset else nc.vector
                        vms.memset(Vb[:, :, 0:PAD], BIG)
                        vms.memset(Vb[:, :, PAD + W:PW], BIG)
                        nc.scalar.copy(out=Vb[:, :, PAD:PAD + W], in_=VP[:])
                        # V chain on SBUF: shifts 1,2,4,7
                        cur, wid = Vb, PW
                        for s in (1, 2, 4):
                            nw = wid - s
                            nxt = vwork_pool.tile([P, tpv, PW], BF16,
                                                  tag="vwork")
                            nc.vector.tensor_tensor(
                                out=nxt[:, :, 0:nw], in0=cur[:, :, 0:nw],
                                in1=cur[:, :, s:s + nw], op=MIN,
                            )
                            cur, wid = nxt, nw
                        vm = vwork_pool.tile([P, tpv, W], BF16, tag="vmin")
                        nc.vector.tensor_tensor(
                            out=vm[:], in0=cur[:, :, 0:W],
                            in1=cur[:, :, 7:7 + W], op=MIN,
                        )
                        vmins.append(vm)

                    # ---- transpose back + cast to f32 + store ----
                    # stores issue from the Act engine queue (TRN2 HWDGE
                    # engines are SP + Act) so a blocked input load on SP
                    # can't head-of-line-block the stores.
                    st = nc.scalar if store_eng == "scalar" else nc.sync
                    yr = y[b, 0].rearrange("(i p) w -> p i w", p=P)
                    OUT = out_pool.tile([P, NT, W], F32, tag="outp")
                    for i in range(NT):
                        TO = tout_pool.tile([P, W], BF16, tag="to")
                        for j in range(NT):
                            vg = vmins[j // tpv]
                            nc.tensor.transpose(
                                TO[:, j * P:(j + 1) * P],
                                vg[:, j % tpv, i * P:(i + 1) * P],
                                ident,
                            )
                        nc.scalar.copy(out=OUT[:, i], in_=TO[:])
                        if split_store:
                            st.dma_start(out=yr[:, i], in_=OUT[:, i])
                    if not split_store:
                        st.dma_start(out=yr, in_=OUT[:])
    nc.compile()
    return nc


_CACHE = {}


def _get_nc(**kw):
    key = tuple(sorted(kw.items()))
    if key not in _CACHE:
        _CACHE[key] = _build(**kw)
    return _CACHE[key]


def kernel(x: np.ndarray) -> np.ndarray:
    """Full-input entry point: x [16,3,512,512] f32 -> [16,1,512,512] f32."""
    x = np.ascontiguousarray(x, dtype=np.float32)
    B = x.shape[0]
    assert B == N_CORES * B_PER_CORE, x.shape
    nc = _get_nc()
    in_maps = [
        {"x": x[c * B_PER_CORE:(c + 1) * B_PER_CORE]} for c in range(N_CORES)
    ]
    res = run_bass_kernel_spmd(nc, in_maps, core_ids=list(range(N_CORES)))
    out = np.concatenate([res.results[c]["y"] for c in range(N_CORES)], axis=0)
    return out.astype(np.float32, copy=False)
